# revision 41
# baseline (speedup 1.0000x reference)
"""Trainium2 Bass kernel for nn_DenseContrastLoss.

Strategy (data-parallel over instances, 8 cores x 13 instances):
  - Host: gather the 96 sampled pixel-vectors per instance (32 anchor +
    32 pos + 32 neg; indices are host-known) from feats, transpose to
    channel-major, convert to bf16, ship [3 chunks x 128 x 2 x 416] per
    core plus bf16 weights.
  - Device (per core), software-pipelined across 3 chunks so the
    in-order PE queue never waits on the DVE/ACT chain:
      L1 (bf16 matmuls, PSUM-bank alternated) -> relu+b1 (DVE)
      L2 (bf16) -> qs = Square(pp+b2) (ACT, f32r)
      colsum matmul -> ln (ACT) -> broadcast matmul -> rn = exp(-ln/2)
      (ACT, [128,chunk] bf16) -> pn = (pp+b2)*rn (DVE
      scalar_tensor_tensor, straight from PSUM)
    then 13 similarity matmuls col-tiled into one [96, 5*64] PSUM tile
    (instance n -> partition block n%3, col group n//3), and a max-free
    InfoNCE chain: term = ln(1 + sum_m' exp(an) * exp(-ap)) (the
    reference's max-subtraction cancels algebraically), finished by a
    [96,5]x[96,3] block-sum matmul -> 15 floats out.
  - Host: validity mask from gt_mask areas, masked mean, * LOSS_WEIGHT.
"""

import sys

import numpy as np

if "/opt/trn_rl_repo" not in sys.path:
    sys.path.insert(0, "/opt/trn_rl_repo")

import ml_dtypes

import concourse.bass as bass
import concourse.tile as tile
from concourse import bacc, mybir
from concourse.bass_utils import run_bass_kernel_spmd

F32 = mybir.dt.float32
F32R = mybir.dt.float32r
BF16 = mybir.dt.bfloat16

TAU = 0.07
LOSS_WEIGHT = 1.2
NUM_SAMPLES = 32
C = 256
SIDE = 28
PIX = SIDE * SIDE  # 784
N_INST = 100
N_CORES = 8
NI = 13                      # instances per core (8*13 = 104 >= 100)
SAMP = 3 * NUM_SAMPLES       # 96 sampled pixels per instance
STOT = NI * SAMP             # 1248
# uneven chunks (each <= 512, the PSUM fp32 bank limit): a small last
# chunk shortens the serial norm->sim->loss tail
CHUNKS = [512, 512, 224]
COFF = [0, 512, 1024]
NCH = len(CHUNKS)
# instances fully covered once chunk ch's pn is written
SIMS_AT = [(0, 5), (5, 10), (10, 13)]
NJ, NG = 3, 5                # sim packing: partition blocks x col groups
NWARM = 10                   # PE warm-up matmuls during the input DMA


def _build_nc():
    nc = bacc.Bacc("TRN2", target_bir_lowering=False)
    gts = [nc.declare_dram_parameter(f"gt{ch}", [128, 2, CHUNKS[ch]], BF16,
                                     isOutput=False)
           for ch in range(NCH)]
    wcm = nc.declare_dram_parameter("wcm", [128, 2, 2, C], BF16,
                                    isOutput=False)
    bcm = nc.declare_dram_parameter("bcm", [128, 8], F32, isOutput=False)
    # per-(anchor, pos) loss terms ln(1 + S_k exp(-ap)); host reduces
    loss = nc.declare_dram_parameter("loss", [96, NG * 32], F32,
                                     isOutput=True)

    AT = mybir.ActivationFunctionType
    ALU = mybir.AluOpType
    PSUM = bass.MemorySpace.PSUM

    with tile.TileContext(nc) as tc:
        with tc.tile_pool(name="singles", bufs=1) as singles:
            # weights + gather chunks on the sync ring (weights first:
            # they gate the first LDWEIGHTS); tiny biases on the ACT ring
            WC = singles.tile([128, 2, 2, C], BF16)
            nc.sync.dma_start(out=WC[:], in_=wcm[:, :, :, :])
            gch = [singles.tile([128, 2, CHUNKS[ch]], BF16, name=f"g{ch}")
                   for ch in range(NCH)]
            for ch in range(NCH):
                nc.sync.dma_start(out=gch[ch][:], in_=gts[ch][:, :, :])
            BC = singles.tile([128, 8], F32)
            nc.scalar.dma_start(out=BC[:], in_=bcm[:, :])
            # ACT table set (covers exp/ln/square/relu/copy), after the
            # ACT-ring DMA issues but well before the first activation
            nc.scalar.add_instruction(
                mybir.InstLoadActFuncSet(
                    name=nc.get_next_instruction_name(),
                    ins=[],
                    outs=[],
                    act_func_set_id=6,  # natural_log_exp_and_others
                )
            )
            W1, W2 = WC[:, 0], WC[:, 1]
            B1, B2, blk4 = BC[:, 0:2], BC[:, 2:4], BC[:, 4:8]

            onesrf = singles.tile([1, 128], F32)
            nc.vector.memset(onesrf[:], 1.0)
            onescf = singles.tile([128, 1], F32)
            nc.vector.memset(onescf[:], 1.0)
            onesr = singles.tile([1, 128], F32R)
            nc.scalar.copy(out=onesr[:], in_=onesrf[:])
            onesc = singles.tile([128, 1], F32R)
            nc.scalar.copy(out=onesc[:], in_=onescf[:])

            with tc.tile_pool(name="big", bufs=1) as big:
                hs = [big.tile([128, STOT], BF16, name=f"hs{m}")
                      for m in range(2)]
                pn = [big.tile([128, STOT], BF16, name=f"pn{m}")
                      for m in range(2)]

                with (
                    tc.tile_pool(name="mmp", bufs=6, space=PSUM) as mmp,
                    tc.tile_pool(name="nsqp", bufs=1, space=PSUM) as nsqp,
                    tc.tile_pool(name="simp", bufs=1, space=PSUM) as simp,
                    tc.tile_pool(name="qsp", bufs=4) as qsp,
                ):
                    # one bank: sims in cols 0-319, final loss in 320-322
                    sim = simp.tile([96, 512], F32, tag="sim")

                    # PE warm-up during the input-DMA window: ramps HAM to
                    # K=8/8 before the real matmuls arrive.  Writes land in
                    # the sim bank and are overwritten later (start=True).
                    warm = singles.tile([128, 416], BF16, name="warm")
                    nc.vector.memset(warm[:], 1.0)
                    for _ in range(NWARM):
                        nc.tensor.matmul(
                            sim[:96, :320], warm[:, :96], warm[:, :320],
                            start=True, stop=True,
                        )

                    hp = {}
                    pp = {}
                    qs = {}
                    lnt = {}
                    rre = {}

                    def l1(ch):
                        cw = CHUNKS[ch]
                        hp[ch] = [mmp.tile([128, 512], F32, tag="mm",
                                           name=f"hp{ch}_{m}")
                                  for m in range(2)]
                        for k in range(2):
                            for m in range(2):
                                nc.tensor.matmul(
                                    hp[ch][m][:, :cw],
                                    W1[:, k, 128 * m : 128 * (m + 1)],
                                    gch[ch][:, k, :],
                                    start=(k == 0),
                                    stop=(k == 1),
                                )

                    def relu(ch):
                        sl = slice(COFF[ch], COFF[ch] + CHUNKS[ch])
                        for m in range(2):
                            nc.vector.tensor_scalar(
                                out=hs[m][:, sl], in0=hp[ch][m][:, :CHUNKS[ch]],
                                scalar1=B1[:, m : m + 1], scalar2=0.0,
                                op0=ALU.add, op1=ALU.max,
                            )

                    def l2(ch):
                        sl = slice(COFF[ch], COFF[ch] + CHUNKS[ch])
                        pp[ch] = [mmp.tile([128, 512], F32, tag="mm",
                                           name=f"pp{ch}_{m}")
                                  for m in range(2)]
                        for k in range(2):
                            for m in range(2):
                                nc.tensor.matmul(
                                    pp[ch][m][:, :CHUNKS[ch]],
                                    W2[:, k, 128 * m : 128 * (m + 1)],
                                    hs[k][:, sl],
                                    start=(k == 0),
                                    stop=(k == 1),
                                )

                    def sq(ch):
                        qs[ch] = []
                        for m in range(2):
                            q = qsp.tile([128, 512], F32R, tag="qs",
                                         name=f"qs{ch}_{m}")
                            nc.scalar.activation(
                                out=q[:, :CHUNKS[ch]],
                                in_=pp[ch][m][:, :CHUNKS[ch]],
                                func=AT.Square,
                                bias=B2[:, m : m + 1],
                            )
                            qs[ch].append(q)

                    nsqs = {}

                    def colsum(ch):
                        # PE: nsq = ones^T (qs0 | qs1)
                        cw = CHUNKS[ch]
                        nsq = nsqp.tile([1, 512], F32, tag="nsq")
                        for m in range(2):
                            nc.tensor.matmul(
                                nsq[:, :cw], onesc[:], qs[ch][m][:, :cw],
                                start=(m == 0), stop=(m == 1),
                            )
                        nsqs[ch] = nsq

                    def ln_op(ch):
                        cw = CHUNKS[ch]
                        t = big.tile([1, 512], F32R, tag="lnt",
                                     name="lnt", bufs=2)
                        nc.scalar.activation(
                            out=t[:, :cw], in_=nsqs[ch][:, :cw], func=AT.Ln,
                            scale=float(TAU),
                        )
                        lnt[ch] = t

                    def rrep_mm(ch):
                        # PE: broadcast ln row to 128 partitions
                        cw = CHUNKS[ch]
                        r = mmp.tile([128, 512], F32, tag="mm",
                                     name=f"rr{ch}")
                        nc.tensor.matmul(
                            r[:, :cw], onesr[:], lnt[ch][:, :cw],
                            start=True, stop=True,
                        )
                        rre[ch] = r

                    def rn_exp(ch):
                        # ACT: rn = exp(-0.5*ln(tau*nsq)), bf16 [128,chunk]
                        cw = CHUNKS[ch]
                        e = big.tile([128, 512], BF16, tag="rre",
                                     name="rre", bufs=2)
                        nc.scalar.activation(
                            out=e[:, :cw], in_=rre[ch][:, :cw], func=AT.Exp,
                            scale=-0.5,
                        )
                        rre[ch] = e

                    def pnorm(ch):
                        # DVE: pn = (pp + b2) * rn, straight from PSUM
                        cw = CHUNKS[ch]
                        sl = slice(COFF[ch], COFF[ch] + cw)
                        for m in range(2):
                            nc.vector.scalar_tensor_tensor(
                                out=pn[m][:, sl], in0=pp[ch][m][:, :cw],
                                scalar=B2[:, m : m + 1], in1=rre[ch][:, :cw],
                                op0=ALU.add, op1=ALU.mult,
                            )

                    def sims(n0, n1, kgrouped=False):
                        # kgrouped: issue all k=0 matmuls first so they can
                        # start as soon as pn[0]'s chunk lands
                        korder = ([(k, n) for k in range(2)
                                   for n in range(n0, n1)]
                                  if kgrouped else
                                  [(k, n) for n in range(n0, n1)
                                   for k in range(2)])
                        for k, n in korder:
                            a0 = SAMP * n
                            j, g = n % NJ, n // NJ
                            dst = sim[32 * j : 32 * (j + 1),
                                      64 * g : 64 * (g + 1)]
                            nc.tensor.matmul(
                                dst,
                                pn[k][:, a0 : a0 + 32],
                                pn[k][:, a0 + 32 : a0 + 96],
                                start=(k == 0),
                                stop=(k == 1),
                            )

                    # ---- max-free InfoNCE chain, split by col groups so
                    # the first part overlaps the last chunk's norm ----
                    sim3 = sim[:, : NG * 64].rearrange(
                        "p (g m) -> p g m", g=NG
                    )
                    ee = big.tile([96, NG * 32], F32, name="ee")
                    s4 = big.tile([96, NG], F32, name="s4")
                    em = big.tile([96, NG * 32], F32, name="em")
                    tt = big.tile([96, NG * 32], F32, name="tt")
                    ctb = big.tile([96, NG * 32], F32, name="ctb")

                    def chain(g0, g1):
                        gn = g1 - g0
                        c0, c1 = 32 * g0, 32 * g1
                        ee3 = ee[:, c0:c1].rearrange("p (g m) -> p g m", g=gn)
                        em3 = em[:, c0:c1].rearrange("p (g m) -> p g m", g=gn)
                        nc.scalar.activation(
                            out=ee3, in_=sim3[:, g0:g1, 32:64], func=AT.Exp,
                        )
                        nc.vector.reduce_sum(
                            out=s4[:, g0:g1], in_=ee3,
                            axis=mybir.AxisListType.X,
                        )
                        nc.scalar.activation(
                            out=em3, in_=sim3[:, g0:g1, 0:32], func=AT.Exp,
                            scale=-1.0,
                        )
                        nc.vector.tensor_mul(
                            out=tt[:, c0:c1].rearrange(
                                "p (g m) -> p g m", g=gn
                            ),
                            in0=em3,
                            in1=s4[:, g0:g1].unsqueeze(-1).broadcast_to(
                                [96, gn, 32]
                            ),
                        )
                        nc.scalar.activation(
                            out=ctb[:, c0:c1], in_=tt[:, c0:c1], func=AT.Ln,
                            bias=1.0,
                        )

                    # ---- software-pipelined issue order ----
                    l1(0); relu(0)
                    l1(1); relu(1)
                    l1(2); relu(2)
                    l2(0); sq(0); colsum(0); ln_op(0)
                    l2(1); sq(1); rrep_mm(0); rn_exp(0); pnorm(0)
                    colsum(1)
                    l2(2); ln_op(1); sq(2)
                    rrep_mm(1); rn_exp(1); pnorm(1)
                    sims(0, 5); sims(5, 10)
                    colsum(2); ln_op(2); rrep_mm(2); rn_exp(2); pnorm(2)
                    chain(0, 3)          # instances 0-8, overlaps chunk 2
                    nc.sync.dma_start(out=loss[:, :96], in_=ctb[:, :96])
                    sims(10, NI)
                    chain(3, NG)         # instances 9-12
                    # garbage in the two unused slots (g=4, j=1,2) stays
                    # in its blocks; the host only reads valid ones
                    nc.sync.dma_start(out=loss[:, 96:], in_=ctb[:, 96:])

    nc.compile()
    return nc


_NC_CACHE = None


def _get_nc():
    global _NC_CACHE
    if _NC_CACHE is None:
        _NC_CACHE = _build_nc()
    return _NC_CACHE


def _host_prep(feats, w1, b1, w2, b2, anchor_inds, pos_inds, neg_inds):
    """Build the 8 per-core input maps."""
    n = feats.shape[0]
    ntot = N_CORES * NI
    ff = np.asarray(feats, dtype=np.float32).reshape(n, C, PIX)

    def flat(inds):
        inds = np.asarray(inds)
        f = inds[..., 0].astype(np.int64) * SIDE + inds[..., 1].astype(np.int64)
        if ntot > n:
            f = np.concatenate(
                [f, np.broadcast_to(f[0], (ntot - n,) + f.shape[1:])], axis=0
            )
        return f  # [ntot, 32]

    af, pf, nf = flat(anchor_inds), flat(pos_inds), flat(neg_inds)
    samp = np.concatenate([af, pf, nf], axis=1)  # [ntot, 96]
    idx = np.minimum(np.arange(ntot), n - 1)
    g = np.take_along_axis(ff[idx], samp[:, None, :], axis=2)
    # per-core channel-major [C, 1248] -> bf16 chunks [128, 2, cw]
    g = g.reshape(N_CORES, NI, C, SAMP)
    g = np.transpose(g, (0, 2, 1, 3)).reshape(N_CORES, C, STOT)
    g = g.astype(ml_dtypes.bfloat16)
    g = g.reshape(N_CORES, 2, 128, STOT)  # c = k*128 + p
    g = np.transpose(g, (0, 2, 1, 3))     # [cores, 128, 2, STOT]
    gchunks = [
        np.ascontiguousarray(g[:, :, :, COFF[ch] : COFF[ch] + CHUNKS[ch]])
        for ch in range(NCH)
    ]

    def wprep(w):
        wt = np.asarray(w, dtype=np.float32).T  # [c, d]
        wt = wt.reshape(2, 128, C)              # [k, p, d]
        return np.transpose(wt, (1, 0, 2)).astype(ml_dtypes.bfloat16)

    wcm = np.ascontiguousarray(
        np.stack([wprep(w1), wprep(w2)], axis=1)
    )  # [128, 2, 2, C]
    bcm = np.zeros((128, 8), dtype=np.float32)
    bcm[:, 0:2] = np.asarray(b1, dtype=np.float32).reshape(2, 128).T
    bcm[:, 2:4] = np.asarray(b2, dtype=np.float32).reshape(2, 128).T
    for j in range(4):
        bcm[32 * j : 32 * (j + 1), 4 + j] = 1.0

    in_maps = []
    for c in range(N_CORES):
        m = {"wcm": wcm, "bcm": bcm}
        for ch in range(NCH):
            m[f"gt{ch}"] = gchunks[ch][c]
        in_maps.append(m)
    return in_maps


def _finalize(loss_per, gt_mask):
    gt = np.asarray(gt_mask)
    area = gt.reshape(gt.shape[0], -1).sum(axis=1)
    valid = (area > NUM_SAMPLES) & (area < PIX - NUM_SAMPLES)
    n_valid = np.float32(valid.sum())
    if n_valid > 0:
        total = np.float32(np.where(valid, loss_per, 0.0).astype(np.float32).sum())
        out = total / max(n_valid, np.float32(1.0))
    else:
        out = np.float32(0.0)
    return np.float32(out * np.float32(LOSS_WEIGHT))


def kernel(feats, w1, b1, w2, b2, gt_mask, anchor_inds, pos_inds, neg_inds,
           _results_hook=None):
    nc = _get_nc()
    in_maps = _host_prep(feats, w1, b1, w2, b2, anchor_inds, pos_inds, neg_inds)
    res = run_bass_kernel_spmd(nc, in_maps, list(range(N_CORES)))
    if _results_hook is not None:
        _results_hook(res)
    parts = []
    for c in range(N_CORES):
        ctb = res.results[c]["loss"]  # [96, NG*32]
        for n in range(NI):
            j, g = n % NJ, n // NJ
            blk = ctb[32 * j : 32 * (j + 1), 32 * g : 32 * (g + 1)]
            parts.append(blk.sum(dtype=np.float32))
    loss_per = np.array(parts, dtype=np.float32)[: N_INST] / np.float32(
        NUM_SAMPLES * NUM_SAMPLES
    )
    return _finalize(loss_per, gt_mask)
